# revision 18
# baseline (speedup 1.0000x reference)
"""Trainium2 Bass kernel for a GPT-style transformer block (B=2,T=2048,C=768,H=12).

Sharding: 8 cores; core c handles batch b=c//4, query block qo=(c%4)*512.
Feature-major [C,T] per core, rolled so its 512 query tokens are cols 0:512.
K/V computed for all 2048 keys per core (no cross-core communication).

Numerics: fp8e4m3 DoubleRow matmuls for QKV/AV/FC/proj (weights pre-scaled by
per-matrix powers of two on host, descaled on-device at copyback/exp/gelu),
bf16 scores + Wo, fp32/f32r residual path.

Structure: LN stats via DVE reciprocal + ScalarE Sqrt (no Ln/Exp table
thrash); scores computed as row-tiled head pairs (even head rows 0:64, odd
rows 64:128 run concurrently on the PE); V projection and per-head softmax
denominators/Wo interleaved into the exp-bound attention window; MLP hidden
pairs feed DoubleRow proj accumulation.
"""
import sys

sys.path.insert(0, "/opt/trn_rl_repo")

import numpy as np
import ml_dtypes

import concourse.bass as bass
import concourse.tile as tile
from concourse import bacc, mybir
from concourse.bass_utils import run_bass_kernel_spmd

F32 = mybir.dt.float32
F32R = mybir.dt.float32r
BF16 = mybir.dt.bfloat16
FP8 = mybir.dt.float8e4
AF = mybir.ActivationFunctionType
ALU = mybir.AluOpType
DR = mybir.MatmulPerfMode.DoubleRow

B, T, C, H = 2, 2048, 768, 12
HD = C // H             # 64
C4 = 4 * C              # 3072
EPS = 1e-5
NCORES = 8
TQ = (B * T) // NCORES  # 512
PC = C // 128           # 6
PC4 = C4 // 128         # 24
NT4 = T // 512          # 4
NSC = T // 128          # 16
VP = 80                 # padded per-head vt stride (64 v dims + 1 ones + pad)
NBIAS = PC4             # bfc bias columns for gelu


def _build(flags, scales, reps=1):
    (has_qkv_bias, has_o_bias, has_proj_bias, has_fc_bias, has_mask) = flags
    (eq, ek, ev, efc, ep) = scales
    nc = bacc.Bacc()

    x_d = nc.dram_tensor("x_fm", [C, T], F32, kind="ExternalInput")
    xb_d = nc.dram_tensor("x_bf", [C, T], BF16, kind="ExternalInput")
    wq_d = nc.dram_tensor("wq8", [128, PC // 2, 2, C], FP8, kind="ExternalInput")
    wk_d = nc.dram_tensor("wk8", [128, PC // 2, 2, C], FP8, kind="ExternalInput")
    wv_d = nc.dram_tensor("wv8", [128, PC // 2, 2, C], FP8, kind="ExternalInput")
    wo_d = nc.dram_tensor("wo", [128, H // 2, C], BF16, kind="ExternalInput")
    wfc_d = nc.dram_tensor("wfc8", [PC4, 128, PC // 2, 2, 128], FP8,
                           kind="ExternalInput")
    wproj_d = nc.dram_tensor("wproj8", [PC4 // 2, 128, 2, C], FP8,
                             kind="ExternalInput")
    bias_d = nc.dram_tensor("biases", [128, NBIAS], F32, kind="ExternalInput")
    brow_d = nc.dram_tensor("bias_rows", [1, 5 * C], F32R, kind="ExternalInput")
    mask_d = nc.dram_tensor("maskb", [128, NSC], F32, kind="ExternalInput")
    out_d = nc.dram_tensor("out_fm", [C, TQ], F32, kind="ExternalOutput")

    x_pot = x_d.rearrange("(o p) t -> p o t", p=128)
    xb_pot = xb_d.rearrange("(o p) t -> p o t", p=128)
    out_pot = out_d.rearrange("(o p) t -> p o t", p=128)

    exp_scale = float(2.0 ** (-(eq + ek)))
    vcol_scale = float(2.0 ** (-2 * ev))     # folded into istdv via sqrt(scale*x)
    gelu_scale = float(2.0 ** (-efc))
    proj_scale = float(2.0 ** (-ep))

    with tile.TileContext(nc) as tc:
      for _rep in range(reps):
        with tc.tile_pool(name=f"const{_rep}", bufs=1) as const, \
             tc.tile_pool(name=f"persist{_rep}", bufs=1) as persist:

            # ---------------- constants ----------------
            ones_f = const.tile([128, 1], F32)
            nc.vector.memset(ones_f[:], 1.0)
            ones_col_b = const.tile([128, 1], BF16)
            nc.vector.memset(ones_col_b[:], 1.0)
            ones_col_r = const.tile([128, 1], F32R)
            nc.vector.tensor_copy(ones_col_r[:], ones_f[:])
            ones2d_f = const.tile([128, 128], F32)
            nc.vector.memset(ones2d_f[:], 1.0)
            ones2d_r = const.tile([128, 128], F32R)
            nc.vector.tensor_copy(ones2d_r[:], ones2d_f[:])
            negones_r = const.tile([1, 128], F32R)
            nc.vector.tensor_scalar_mul(negones_r[:], ones2d_f[0:1, :], -1.0)
            onesr512_f = const.tile([1, TQ], F32)
            nc.vector.memset(onesr512_f[:], 1.0)
            ones_row512 = const.tile([1, TQ], F32R)
            nc.vector.tensor_copy(ones_row512[:], onesr512_f[:])
            ones_hn = const.tile([128, H * NSC], BF16)
            nc.vector.memset(ones_hn[:], 1.0)

            if has_fc_bias:
                bias_sb = const.tile([128, NBIAS], F32)
                nc.sync.dma_start(bias_sb[:], bias_d[:, :])
            if has_qkv_bias or has_o_bias or has_proj_bias:
                brow_sb = const.tile([1, 5 * C], F32R)
                nc.sync.dma_start(brow_sb[:], brow_d[:, :])
            if has_mask:
                mask_sb = const.tile([128, NSC], F32)
                nc.sync.dma_start(mask_sb[:], mask_d[:, :])

            x_q = persist.tile([128, PC, TQ], F32)    # exact residual copy
            x2 = persist.tile([128, PC, TQ], F32R)    # post-attention residual
            xn2 = persist.tile([128, PC, TQ], FP8)    # normalized x2 for MLP
            out_sb = persist.tile([128, PC, TQ], F32)

            with tc.tile_pool(name=f"abc{_rep}", bufs=1) as abc:
                x8 = abc.tile([128, PC, T], FP8)       # centered x, fp8
                q_bf = abc.tile([128, PC, TQ], BF16)
                k_bf = abc.tile([128, PC, T], BF16)
                vt = abc.tile([128, H, NSC, VP], FP8)  # token-major V + ones col
                y_nm = abc.tile([128, H // 2, TQ], BF16)
                istd_b = abc.tile([128, T], BF16)
                istd_col = abc.tile([128, NSC], F32)   # istd*2^-ev token-major
                nrows = 12 if has_qkv_bias else 8
                rows = abc.tile([1, nrows, 512], F32R)  # mean/istd/risd rows
                dn = abc.tile([128, 2, TQ], F32R)      # recip rows at lane 64

                wq_sb = abc.tile([128, PC // 2, 2, C], FP8)
                wk_sb = abc.tile([128, PC // 2, 2, C], FP8)
                wv_sb = abc.tile([128, PC // 2, 2, C], FP8)
                wo_sb = abc.tile([128, H // 2, C], BF16)

                # ======== phase A: LN1 stats + centering ========
                with tc.tile_pool(name=f"xbf{_rep}", bufs=1) as xbfp, \
                     tc.tile_pool(name=f"sqp{_rep}", bufs=3) as sqp, \
                     tc.tile_pool(name=f"rtmp{_rep}", bufs=2) as rtmp, \
                     tc.tile_pool(name=f"nmsb{_rep}", bufs=2) as nmsb:
                  x_bf = xbfp.tile([128, PC, T], BF16)
                  for t4 in range(NT4):
                      sl = slice(t4 * 512, (t4 + 1) * 512)
                      nc.sync.dma_start(x_bf[:, :, sl], xb_pot[:, :, sl])
                      if t4 == 0:
                          nc.sync.dma_start(wq_sb[:], wq_d[:, :, :, :])
                      if t4 == 1:
                          nc.sync.dma_start(wk_sb[:], wk_d[:, :, :, :])
                          nc.sync.dma_start(wv_sb[:], wv_d[:, :, :, :])
                      if t4 == 2:
                          nc.sync.dma_start(wo_sb[:], wo_d[:, :, :])
                          nc.sync.dma_start(x_q[:], x_pot[:, :, 0:TQ])

                  with tc.tile_pool(name=f"st_ps{_rep}", bufs=2,
                                    space="PSUM") as st_ps:
                    for t4 in range(NT4):
                      sl = slice(t4 * 512, (t4 + 1) * 512)
                      p1 = st_ps.tile([1, 512], F32, tag="p1")
                      for j in range(PC):
                          nc.tensor.matmul(p1[:], ones_col_b[:], x_bf[:, j, sl],
                                           start=(j == 0), stop=(j == PC - 1))
                      p2 = st_ps.tile([1, 512], F32, tag="p2")
                      for j in range(PC):
                          xsq = sqp.tile([128, 512], BF16, tag="xsq")
                          nc.vector.tensor_mul(xsq[:], x_bf[:, j, sl],
                                               x_bf[:, j, sl])
                          nc.tensor.matmul(p2[:], ones_col_b[:], xsq[:],
                                           start=(j == 0), stop=(j == PC - 1))
                      mean_r = rows[:, 0 + t4, :]
                      nc.vector.tensor_scalar_mul(mean_r, p1[:], 1.0 / C)
                      msq = rtmp.tile([1, 512], F32, tag="msq")
                      nc.vector.tensor_tensor(msq[:], mean_r, mean_r, ALU.mult)
                      var_r = rtmp.tile([1, 512], F32, tag="var")
                      nc.vector.tensor_scalar(var_r[:], p2[:], 1.0 / C, EPS,
                                              ALU.mult, ALU.add)
                      nc.vector.tensor_sub(var_r[:], var_r[:], msq[:])
                      rvar = rtmp.tile([1, 512], F32, tag="rvar")
                      nc.vector.reciprocal(rvar[:], var_r[:])
                      istd_r = rows[:, 4 + t4, :]
                      nc.scalar.activation(istd_r, rvar[:], AF.Sqrt)
                      istdv_r = rtmp.tile([1, 512], F32, tag="iv")
                      nc.scalar.activation(istdv_r[:], rvar[:], AF.Sqrt,
                                           scale=vcol_scale)
                      if has_qkv_bias:
                          risd_r = rows[:, 8 + t4, :]
                          nc.scalar.activation(risd_r, var_r[:], AF.Sqrt)
                      for o in range(4):
                          cc = t4 * 4 + o
                          nc.sync.dma_start(istd_col[:, cc:cc + 1],
                                            istdv_r[0:1, o * 128:(o + 1) * 128])
                      # broadcasts: -mean and istd across partitions
                      nm_ps = st_ps.tile([128, 512], F32, tag="nm")
                      nc.tensor.matmul(nm_ps[:], negones_r[:], mean_r,
                                       start=True, stop=True)
                      bp = st_ps.tile([128, 512], F32, tag="bp")
                      nc.tensor.matmul(bp[:], ones2d_r[0:1, :], istd_r,
                                       start=True, stop=True)
                      nm_sb = nmsb.tile([128, 512], BF16, tag="nmsb")
                      nc.vector.tensor_copy(nm_sb[:], nm_ps[:])
                      nc.vector.tensor_copy(istd_b[:, sl], bp[:])
                      for j in range(PC):
                          eng = nc.gpsimd if (j % 2) else nc.vector
                          eng.tensor_tensor(x8[:, j, sl], x_bf[:, j, sl],
                                            nm_sb[:], ALU.add)

                # ======== phase B: Q/K projections (fp8 DoubleRow) ========
                with tc.tile_pool(name=f"qp_ps{_rep}", bufs=2,
                                  space="PSUM") as qp_ps, \
                     tc.tile_pool(name=f"kp_ps{_rep}", bufs=2,
                                  space="PSUM") as kp_ps:
                    for oc in range(PC):
                        osl = slice(oc * 128, (oc + 1) * 128)
                        qp = qp_ps.tile([128, 512], F32, tag="qp")
                        for jp in range(PC // 2):
                            nc.tensor.matmul(
                                qp[:], wq_sb[:, jp, :, osl],
                                x8[:, 2 * jp:2 * jp + 2, 0:TQ],
                                start=(jp == 0),
                                stop=(jp == PC // 2 - 1 and not has_qkv_bias),
                                perf_mode=DR)
                        if has_qkv_bias:
                            nc.tensor.matmul(qp[:], brow_sb[:, osl],
                                             rows[:, 8, :],
                                             start=False, stop=True)
                        nc.vector.tensor_tensor(q_bf[:, oc, :], qp[:],
                                                istd_b[:, 0:TQ], ALU.mult)
                    for oc in range(PC):
                        osl = slice(oc * 128, (oc + 1) * 128)
                        for t4p in range(NT4 // 2):
                            kp = kp_ps.tile([128, 2, 512], F32, tag="kp")
                            for i in range(2):
                                t4 = 2 * t4p + i
                                sl = slice(t4 * 512, (t4 + 1) * 512)
                                for jp in range(PC // 2):
                                    nc.tensor.matmul(
                                        kp[:, i, :], wk_sb[:, jp, :, osl],
                                        x8[:, 2 * jp:2 * jp + 2, sl],
                                        start=(jp == 0),
                                        stop=(jp == PC // 2 - 1 and not has_qkv_bias),
                                        perf_mode=DR)
                                if has_qkv_bias:
                                    nc.tensor.matmul(
                                        kp[:, i, :], brow_sb[:, C + osl.start:C + osl.stop],
                                        rows[:, 8 + t4, :],
                                        start=False, stop=True)
                            sl2 = slice(t4p * 1024, (t4p + 1) * 1024)
                            nc.vector.tensor_tensor(
                                k_bf[:, oc, sl2].rearrange("p (a t) -> p a t", a=2),
                                kp[:],
                                istd_b[:, sl2].rearrange("p (a t) -> p a t", a=2),
                                ALU.mult)

                # ======== phase C: attention (V + scores/exp/AV + Wo) ========
                nc.vector.tensor_copy(
                    vt[:, :, :, HD],
                    ones_hn[:].rearrange("p (h s) -> p h s", h=H))
                with tc.tile_pool(name=f"sc_ps{_rep}", bufs=2, space="PSUM") as sc_ps, \
                     tc.tile_pool(name=f"yp_ps{_rep}", bufs=1, space="PSUM") as yp_ps, \
                     tc.tile_pool(name=f"vp_ps{_rep}", bufs=1, space="PSUM") as vp_ps, \
                     tc.tile_pool(name=f"rp_ps{_rep}", bufs=1, space="PSUM") as rp_ps, \
                     tc.tile_pool(name=f"attb{_rep}", bufs=2) as attb:
                    for hp in range(H // 2):
                        ch = hp
                        yp_e = yp_ps.tile([HD + 1, TQ], F32, tag="ype")
                        yp_o = yp_ps.tile([HD + 1, TQ], F32, tag="ypo")
                        att = None
                        for sc in range(NSC):
                            ksl = slice(sc * 128, (sc + 1) * 128)
                            # V projection interleaved into first two pairs
                            if hp < 2:
                                half = hp
                                hsl = slice(half * 384, (half + 1) * 384)
                                vp = vp_ps.tile([128, 384], F32, tag="vp")
                                for jp in range(PC // 2):
                                    nc.tensor.matmul(
                                        vp[:], x8[:, 2 * jp:2 * jp + 2, ksl],
                                        wv_sb[:, jp, :, hsl],
                                        start=(jp == 0),
                                        stop=(jp == PC // 2 - 1 and not has_qkv_bias),
                                        perf_mode=DR)
                                if has_qkv_bias:
                                    nc.tensor.matmul(
                                        vp[:], rows[:, 8 + sc // 4,
                                                    (sc % 4) * 128:(sc % 4) * 128 + 128],
                                        brow_sb[:, 2 * C + hsl.start:2 * C + hsl.stop],
                                        start=False, stop=True)
                                dst = vt[:, half * 6:half * 6 + 6, sc, 0:HD]
                                nc.vector.tensor_scalar(
                                    dst, vp[:].rearrange("p (h e) -> p h e", e=HD),
                                    istd_col[:, sc:sc + 1], None, ALU.mult)
                            # scores: row-tiled head pair
                            if sc % 2 == 0:
                                att = attb.tile([128, 2, 2, 512], FP8, tag="att")
                            sp = sc_ps.tile([128, 2, 512], F32, tag="sp")
                            nc.tensor.matmul(sp[:, 0, :], k_bf[0:64, ch, ksl],
                                             q_bf[0:64, ch, :],
                                             start=True, stop=True)
                            nc.tensor.matmul(sp[:, 1, :], k_bf[64:128, ch, ksl],
                                             q_bf[64:128, ch, :],
                                             start=True, stop=True)
                            if has_mask:
                                nc.scalar.activation(att[:, sc % 2, :, :], sp[:],
                                                     AF.Exp, scale=exp_scale,
                                                     bias=mask_sb[:, sc:sc + 1])
                            else:
                                nc.scalar.activation(att[:, sc % 2, :, :], sp[:],
                                                     AF.Exp, scale=exp_scale)
                            if sc % 2 == 1:
                                scp = sc // 2
                                nc.tensor.matmul(
                                    yp_e[:], vt[:, 2 * hp, 2 * scp:2 * scp + 2, 0:HD + 1],
                                    att[:, :, 0, :],
                                    start=(scp == 0), stop=(scp == NSC // 2 - 1),
                                    perf_mode=DR)
                                nc.tensor.matmul(
                                    yp_o[:], vt[:, 2 * hp + 1, 2 * scp:2 * scp + 2, 0:HD + 1],
                                    att[:, :, 1, :],
                                    start=(scp == 0), stop=(scp == NSC // 2 - 1),
                                    perf_mode=DR)
                        # softmax denominators + normalize (per head pair)
                        with nc.allow_low_precision(reason="softmax denom rows"):
                            nc.vector.reciprocal(dn[64:65, 0, :],
                                                 yp_e[HD:HD + 1, :])
                            nc.vector.reciprocal(dn[64:65, 1, :],
                                                 yp_o[HD:HD + 1, :])
                        rp = rp_ps.tile([64, TQ], F32, tag="rp")
                        rp_sb = attb.tile([64, 2, TQ], F32, tag="rpsb")
                        nc.tensor.matmul(rp[:], ones2d_r[64:65, 0:64],
                                         dn[64:65, 0, :], start=True, stop=True)
                        nc.vector.tensor_copy(rp_sb[:, 0, :], rp[:])
                        nc.tensor.matmul(rp[:], ones2d_r[64:65, 0:64],
                                         dn[64:65, 1, :], start=True, stop=True)
                        nc.vector.tensor_copy(rp_sb[:, 1, :], rp[:])
                        nc.vector.tensor_tensor(y_nm[0:64, hp, :], yp_e[0:HD, :],
                                                rp_sb[:, 0, :], ALU.mult)
                        y_tmp = attb.tile([64, TQ], BF16, tag="ytmp")
                        nc.vector.tensor_tensor(y_tmp[:], yp_o[0:HD, :],
                                                rp_sb[:, 1, :], ALU.mult)
                        nc.sync.dma_start(y_nm[64:128, hp, :], y_tmp[:])

                # ---- Wo + residual + LN2 stats ----
                with tc.tile_pool(name=f"wo_ps{_rep}", bufs=2, space="PSUM") as wo_ps, \
                     tc.tile_pool(name=f"st2{_rep}", bufs=1, space="PSUM") as st2, \
                     tc.tile_pool(name=f"dtmp{_rep}", bufs=2) as dtmp:
                    p21 = st2.tile([1, TQ], F32, tag="p21")
                    p22 = st2.tile([1, TQ], F32, tag="p22")
                    for oc in range(PC):
                        osl = slice(oc * 128, (oc + 1) * 128)
                        op_e = wo_ps.tile([128, TQ], F32, tag="ope")
                        op_o = wo_ps.tile([128, TQ], F32, tag="opo")
                        for hp in range(H // 2):
                            nc.tensor.matmul(op_e[:], wo_sb[0:64, hp, osl],
                                             y_nm[0:64, hp, :],
                                             start=(hp == 0),
                                             stop=(hp == H // 2 - 1 and not has_o_bias))
                            nc.tensor.matmul(op_o[:], wo_sb[64:128, hp, osl],
                                             y_nm[64:128, hp, :],
                                             start=(hp == 0),
                                             stop=(hp == H // 2 - 1))
                        if has_o_bias:
                            nc.tensor.matmul(op_e[:], brow_sb[:, 3 * C + osl.start:3 * C + osl.stop],
                                             ones_row512[:], start=False, stop=True)
                        t_oc = dtmp.tile([128, TQ], F32, tag="toc")
                        nc.vector.tensor_tensor(t_oc[:], op_e[:], x_q[:, oc, :],
                                                ALU.add)
                        nc.vector.tensor_tensor(x2[:, oc, :], t_oc[:], op_o[:],
                                                ALU.add)
                        nc.tensor.matmul(p21[:], ones_col_r[:], x2[:, oc, :],
                                         start=(oc == 0), stop=(oc == PC - 1))
                        xsq2 = dtmp.tile([128, TQ], F32R, tag="xsq2")
                        nc.vector.tensor_mul(xsq2[:], x2[:, oc, :], x2[:, oc, :])
                        nc.tensor.matmul(p22[:], ones_col_r[:], xsq2[:],
                                         start=(oc == 0), stop=(oc == PC - 1))
                    # LN2 istd
                    mean2 = rows[:, 0, :]
                    nc.vector.tensor_scalar_mul(mean2, p21[:], 1.0 / C)
                    msq2 = dtmp.tile([1, TQ], F32, tag="m2")
                    nc.vector.tensor_tensor(msq2[:], mean2, mean2, ALU.mult)
                    var2 = dtmp.tile([1, TQ], F32, tag="v2")
                    nc.vector.tensor_scalar(var2[:], p22[:], 1.0 / C, EPS,
                                            ALU.mult, ALU.add)
                    nc.vector.tensor_sub(var2[:], var2[:], msq2[:])
                    rvar2 = dtmp.tile([1, TQ], F32, tag="rv2")
                    nc.vector.reciprocal(rvar2[:], var2[:])
                    istd2 = rows[:, 1, :]
                    nc.scalar.activation(istd2, rvar2[:], AF.Sqrt)
                    nm2_ps = st2.tile([128, TQ], F32, tag="nm2")
                    nc.tensor.matmul(nm2_ps[:], negones_r[:], mean2,
                                     start=True, stop=True)
                    bp2 = st2.tile([128, TQ], F32, tag="bp2")
                    nc.tensor.matmul(bp2[:], ones2d_r[0:1, :], istd2,
                                     start=True, stop=True)
                    nm2_sb = dtmp.tile([128, TQ], F32, tag="nm2s")
                    nc.vector.tensor_copy(nm2_sb[:], nm2_ps[:])
                    istd2_sb = dtmp.tile([128, TQ], F32, tag="is2")
                    nc.vector.tensor_copy(istd2_sb[:], bp2[:])
                    for j in range(PC):
                        eng = nc.gpsimd if (j % 2) else nc.vector
                        tj = dtmp.tile([128, TQ], F32, tag="tj")
                        eng.tensor_tensor(tj[:], x2[:, j, :], nm2_sb[:], ALU.add)
                        eng.tensor_tensor(xn2[:, j, :], tj[:], istd2_sb[:],
                                          ALU.mult)

            # ============ phase D: MLP (fp8 DoubleRow) ============
            with tc.tile_pool(name=f"hcp{_rep}", bufs=1) as hcp, \
                 tc.tile_pool(name=f"w_sb2{_rep}", bufs=6) as w_sb2, \
                 tc.tile_pool(name=f"pr_ps{_rep}", bufs=1, space="PSUM") as pr_ps, \
                 tc.tile_pool(name=f"fc_ps{_rep}", bufs=2, space="PSUM") as fc_ps:
                hc = hcp.tile([128, PC4, TQ], FP8)
                prs = [pr_ps.tile([128, TQ], F32, tag=f"pr{i}", name=f"pr{i}")
                       for i in range(PC)]
                for kc in range(PC4):
                    wfcc = w_sb2.tile([128, PC // 2, 2, 128], FP8, tag="wfcc")
                    nc.sync.dma_start(wfcc[:], wfc_d[kc])
                    fp = fc_ps.tile([128, TQ], F32, tag="fp")
                    for jp in range(PC // 2):
                        nc.tensor.matmul(fp[:], wfcc[:, jp, :, :],
                                         xn2[:, 2 * jp:2 * jp + 2, :],
                                         start=(jp == 0), stop=(jp == PC // 2 - 1),
                                         perf_mode=DR)
                    if has_fc_bias:
                        nc.scalar.activation(hc[:, kc, :], fp[:], AF.Gelu,
                                             scale=gelu_scale,
                                             bias=bias_sb[:, kc:kc + 1])
                    else:
                        nc.scalar.activation(hc[:, kc, :], fp[:], AF.Gelu,
                                             scale=gelu_scale)
                    if kc % 2 == 1:
                        kp2 = kc // 2
                        wpc = w_sb2.tile([128, 2, C], FP8, tag="wpc")
                        nc.sync.dma_start(wpc[:], wproj_d[kp2])
                        for oc in range(PC):
                            osl = slice(oc * 128, (oc + 1) * 128)
                            nc.tensor.matmul(
                                prs[oc][:], wpc[:, :, osl],
                                hc[:, kc - 1:kc + 1, :],
                                start=(kp2 == 0),
                                stop=(kp2 == PC4 // 2 - 1 and not has_proj_bias),
                                perf_mode=DR)
                if has_proj_bias:
                    for oc in range(PC):
                        osl = slice(oc * 128, (oc + 1) * 128)
                        nc.tensor.matmul(prs[oc][:],
                                         brow_sb[:, 4 * C + osl.start:4 * C + osl.stop],
                                         ones_row512[:], start=False, stop=True)
                for oc in range(PC):
                    nc.vector.scalar_tensor_tensor(out_sb[:, oc, :], prs[oc][:],
                                                   proj_scale, x2[:, oc, :],
                                                   ALU.mult, ALU.add)
                    nc.sync.dma_start(out_pot[:, oc, :], out_sb[:, oc, :])

    nc.compile()
    return nc


_CACHE = {}


def _get_program(flags, scales, reps=1):
    key = (flags, scales, reps)
    if key not in _CACHE:
        _CACHE[key] = _build(flags, scales, reps=reps)
    return _CACHE[key]


def _fp8_scale(w):
    m = float(np.abs(w).max())
    if m == 0.0:
        return 0
    return int(np.floor(np.log2(160.0 / m)))


def kernel(**inputs) -> np.ndarray:
    x = np.asarray(inputs["x"], dtype=np.float32)
    padding_mask = np.asarray(inputs["padding_mask"])
    ln1_s = np.asarray(inputs["ln1_scale"], dtype=np.float32)
    ln1_b = np.asarray(inputs["ln1_bias"], dtype=np.float32)
    ln2_s = np.asarray(inputs["ln2_scale"], dtype=np.float32)
    ln2_b = np.asarray(inputs["ln2_bias"], dtype=np.float32)
    Wq = np.asarray(inputs["Wq"], dtype=np.float32)
    Wk = np.asarray(inputs["Wk"], dtype=np.float32)
    Wv = np.asarray(inputs["Wv"], dtype=np.float32)
    bq = np.asarray(inputs["bq"], dtype=np.float32)
    bk = np.asarray(inputs["bk"], dtype=np.float32)
    bv = np.asarray(inputs["bv"], dtype=np.float32)
    Wo = np.asarray(inputs["Wo"], dtype=np.float32)
    bo = np.asarray(inputs["bo"], dtype=np.float32)
    Wfc = np.asarray(inputs["Wfc"], dtype=np.float32)
    bfc = np.asarray(inputs["bfc"], dtype=np.float32)
    Wproj = np.asarray(inputs["Wproj"], dtype=np.float32)
    bproj = np.asarray(inputs["bproj"], dtype=np.float32)

    sc_q = 1.0 / np.sqrt(HD)
    Wq_f = Wq.transpose(1, 0, 2).reshape(C, C)
    Wk_f = Wk.transpose(1, 0, 2).reshape(C, C)
    Wv_f = Wv.transpose(1, 0, 2).reshape(C, C)
    wq_eff = ln1_s[:, None] * Wq_f * sc_q
    wk_eff = ln1_s[:, None] * Wk_f
    wv_eff = ln1_s[:, None] * Wv_f
    wfc_eff = ln2_s[:, None] * Wfc

    eq = _fp8_scale(wq_eff)
    ek = _fp8_scale(wk_eff)
    ev = _fp8_scale(wv_eff)
    efc = _fp8_scale(wfc_eff)
    ep = _fp8_scale(Wproj)

    def to8(w, e):
        return (w * float(2.0 ** e)).astype(ml_dtypes.float8_e4m3)

    # [C, C] -> [128, 3, 2, C] with row c = (2*jp + s)*128 + p
    def qkv_pre(w8):
        return np.ascontiguousarray(
            w8.reshape(PC // 2, 2, 128, C).transpose(2, 0, 1, 3))

    wq8 = qkv_pre(to8(wq_eff, eq))
    wk8 = qkv_pre(to8(wk_eff, ek))
    wv8 = qkv_pre(to8(wv_eff, ev))
    # wfc: [kc][p][jp][s][128] with row c = (2*jp+s)*128 + p
    wfc8 = np.ascontiguousarray(
        to8(wfc_eff, efc).reshape(PC // 2, 2, 128, PC4, 128)
        .transpose(3, 2, 0, 1, 4))
    # wproj: [kp][p][s][C] with hidden row = (2*kp+s)*128 + p
    wproj8 = np.ascontiguousarray(
        to8(Wproj, ep).reshape(PC4 // 2, 2, 128, C).transpose(0, 2, 1, 3))
    # wo: pair-packed [128, 6, C]: row hd + 64*(h%2), slot h//2 = Wo[h*64+hd]
    wo_pre = np.ascontiguousarray(
        Wo.reshape(H // 2, 2, HD, C).transpose(1, 2, 0, 3)
        .reshape(128, H // 2, C)).astype(ml_dtypes.bfloat16)

    bq_eff = ((ln1_b @ Wq_f) * sc_q + bq.reshape(C) * sc_q) * float(2.0 ** eq)
    bk_eff = (ln1_b @ Wk_f + bk.reshape(C)) * float(2.0 ** ek)
    bv_eff = (ln1_b @ Wv_f + bv.reshape(C)) * float(2.0 ** ev)
    bfc_eff = ln2_b @ Wfc + bfc
    brows = np.concatenate(
        [bq_eff, bk_eff, bv_eff, bo, bproj * float(2.0 ** ep)]
    ).astype(np.float32)[None, :]
    bias_pre = np.ascontiguousarray(
        bfc_eff.reshape(NBIAS, 128).T).astype(np.float32)

    has_qkv_bias = bool(np.abs(brows[0, 0:3 * C]).max() > 0)
    has_o_bias = bool(np.abs(bo).max() > 0)
    has_proj_bias = bool(np.abs(bproj).max() > 0)
    has_fc_bias = bool(np.abs(bfc_eff).max() > 0)
    has_mask = bool(padding_mask.any())

    flags = (has_qkv_bias, has_o_bias, has_proj_bias, has_fc_bias, has_mask)
    scales = (eq, ek, ev, efc, ep)
    nc = _get_program(flags, scales)

    shared = {
        "wq8": wq8, "wk8": wk8, "wv8": wv8, "wo": wo_pre,
        "wfc8": wfc8, "wproj8": wproj8,
        "biases": bias_pre, "bias_rows": brows,
    }
    in_maps = []
    for c in range(NCORES):
        b, qo = c // (NCORES // B), (c % (NCORES // B)) * TQ
        xr = np.roll(x[b], -qo, axis=0)
        x_fm = np.ascontiguousarray(xr.T)
        x_bf = x_fm.astype(ml_dtypes.bfloat16)
        mrow = np.roll(padding_mask[b], -qo)
        maskb = np.ascontiguousarray(
            np.where(mrow, -1e30, 0.0).astype(np.float32).reshape(NSC, 128).T)
        in_maps.append({**shared, "x_fm": x_fm, "x_bf": x_bf, "maskb": maskb})

    res = run_bass_kernel_spmd(nc, in_maps, core_ids=list(range(NCORES)))

    out = np.empty((B, T, C), dtype=np.float32)
    for c in range(NCORES):
        b, qo = c // (NCORES // B), (c % (NCORES // B)) * TQ
        out[b, qo:qo + TQ, :] = res.results[c]["out_fm"].T
    return out
